# revision 15
# baseline (speedup 1.0000x reference)
"""3-layer GCN (message passing) on 8 Trainium2 NeuronCores.

Strategy
--------
Per GCN layer (using linearity: gcn(x) = (A_norm @ x) @ W + b):
  1. agg = A_norm @ h      -- sparse aggregate, done as per-dst-block PE matmuls
                              over dma_gather'ed source rows (fp16 table) with
                              host-built one-hot S matrices (segment sum).
  2. h' = ELU(agg @ W + b) -- dense GEMM per 128-node block + ELU epilogue.
Normalization dinv[src]*dinv[dst] is separable: the gather table holds
dinv-prescaled rows; dst-side dinv is applied at PSUM eviction.

Nodes are sharded contiguously across the 8 cores (6250 each); edges assigned
by destination core. Between layers an AllGather republishes the full fp16
node-feature table to every core's HBM for the next layer's gathers.

dma_gather indices are int16 (< 32768), so the 50000-row table is addressed
via two windows: "lo" = rows [0, 32768) of the full table, "hi" = rows
[17232, 50000) (a sliced AP), with flexible assignment of rows in the overlap
so per-(core, block) chunk counts stay uniform across cores (single SPMD
program).
"""

import numpy as np

N = 50000
E = 800000
F = 128
H = 128
O = 64
NCORES = 8
NLOC = N // NCORES           # 6250
P = 128
NBLK = (NLOC + P - 1) // P   # 49, last block has 106 nodes
LAST_ROWS = NLOC - (NBLK - 1) * P   # 106
HI_BASE = N - 32768          # 17232

_CACHE = {}


def _host_prep(x, edge_index):
    """Build per-core gather indices, S matrices, and scale vectors."""
    src = np.ascontiguousarray(edge_index[0]).astype(np.int64)
    dst = np.ascontiguousarray(edge_index[1]).astype(np.int64)
    loops = np.arange(N, dtype=np.int64)
    src = np.concatenate([src, loops])
    dst = np.concatenate([dst, loops])

    deg = np.bincount(dst, minlength=N).astype(np.float64)  # includes self-loop
    dinv = (1.0 / np.sqrt(deg)).astype(np.float32)

    x_t = (dinv[:, None] * np.asarray(x, dtype=np.float32)).astype(np.float16)

    core = dst // NLOC
    ld = dst - core * NLOC
    blk = ld // P
    col = ld - blk * P
    # src class: 0 = must-lo (src < HI_BASE), 1 = flex, 2 = must-hi
    cls = np.where(src < HI_BASE, 0, np.where(src < 32768, 1, 2)).astype(np.int64)

    key = (core * NBLK + blk) * 4 + cls
    order = np.argsort(key, kind="stable")
    src_s = src[order]
    key_s = key[order]
    col_s = col[order]

    cnt = np.bincount(key_s, minlength=NCORES * NBLK * 4).reshape(NCORES, NBLK, 4)
    n_lo = cnt[:, :, 0]
    n_fx = cnt[:, :, 1]
    n_hi = cnt[:, :, 2]

    # uniform (across cores) chunk counts per block: A lo-chunks + B hi-chunks
    A = np.zeros(NBLK, np.int64)
    B = np.zeros(NBLK, np.int64)
    for b in range(NBLK):
        best = None
        a_min = int(np.max(np.ceil(n_lo[:, b] / P)))
        for a in range(a_min, a_min + 3):
            spill = np.maximum(0, n_fx[:, b] - (P * a - n_lo[:, b]))
            bb = int(np.max(np.ceil((n_hi[:, b] + spill) / P)))
            if best is None or a + bb < best[0] + best[1]:
                best = (a, bb)
        A[b], B[b] = best
    C = A + B
    CT = int(C.sum())
    chunk_base = np.concatenate([[0], np.cumsum(C)]).astype(np.int64)
    slot_base = chunk_base * P

    # per-edge slot assignment
    grp = key_s >> 2          # core*NBLK + blk, sorted
    grp_cnt = np.bincount(grp, minlength=NCORES * NBLK)
    grp_start = np.concatenate([[0], np.cumsum(grp_cnt)])
    rank = np.arange(src_s.shape[0]) - grp_start[grp]
    core_s = grp // NBLK
    blk_s = grp % NBLK
    k_lo = np.minimum(n_lo + n_fx, P * A[None, :])      # [NCORES, NBLK]
    k_lo_e = k_lo[core_s, blk_s]
    is_lo = rank < k_lo_e
    slot_in_blk = np.where(is_lo, rank, P * A[blk_s] + (rank - k_lo_e))
    slot = slot_base[blk_s] + slot_in_blk
    idx_val = np.where(is_lo, src_s, src_s - HI_BASE).astype(np.int64)
    assert idx_val.min() >= 0 and idx_val.max() < 32768
    # sanity: hi slots only for idx that are valid in the hi window
    assert np.all(slot_in_blk < P * C[blk_s])

    idx_arrs = []
    S_arrs = []
    for c in range(NCORES):
        m = core_s == c
        sl = slot[m]
        ia = np.zeros(CT * P, np.int16)
        ia[sl] = idx_val[m].astype(np.int16)
        cols = CT * P // 16
        w = np.zeros((cols, 16), np.int16)
        w.reshape(-1)[:] = ia
        idx_arrs.append(np.tile(w.T.copy(), (8, 1)))
        S = np.zeros((CT, P, P), np.float16)
        S[sl // P, sl % P, col_s[m]] = 1.0
        S_arrs.append(np.ascontiguousarray(S.transpose(1, 0, 2)).reshape(P, CT * P))

    dinv_bcast = []
    dinv_blk = []
    for c in range(NCORES):
        loc = np.zeros(NBLK * P, np.float32)
        loc[:NLOC] = dinv[c * NLOC:(c + 1) * NLOC]
        dinv_bcast.append(np.ascontiguousarray(np.broadcast_to(loc[None, :], (P, NBLK * P))))
        dinv_blk.append(np.ascontiguousarray(loc.reshape(NBLK, P).T))

    meta = dict(A=A.tolist(), B=B.tolist(), C=C.tolist(), CT=CT,
                chunk_base=chunk_base.tolist())
    return x_t, idx_arrs, S_arrs, dinv_bcast, dinv_blk, meta


def _build_program(meta):
    import os
    import concourse.mybir as mybir
    import concourse.tile as tile
    from concourse import bacc

    DBG_LAYERS = int(os.environ.get("GCN_LAYERS", "3"))
    DBG_AG = int(os.environ.get("GCN_AG", "1"))
    DBG_BLOCKS = int(os.environ.get("GCN_BLOCKS", str(NBLK)))
    DBG_REPEAT = int(os.environ.get("GCN_REPEAT", "1"))
    DBG_LINGATHER = int(os.environ.get("GCN_LINGATHER", "0"))
    DBG_GATHERONLY = int(os.environ.get("GCN_GATHERONLY", "0"))

    A, B, C = meta["A"], meta["B"], meta["C"]
    CT = meta["CT"]
    chunk_base = meta["chunk_base"]
    dt = mybir.dt
    ALU = mybir.AluOpType
    ACTF = mybir.ActivationFunctionType

    nc = bacc.Bacc("TRN2", target_bir_lowering=False, num_devices=NCORES)

    t_xt = nc.dram_tensor("x_t", [N, F], dt.float16, kind="ExternalInput")
    t_idx = nc.dram_tensor("idx", [P, CT * 8], dt.int16, kind="ExternalInput")
    t_S = nc.dram_tensor("S", [P, CT * P], dt.float16, kind="ExternalInput")
    t_dbc = nc.dram_tensor("dinv_bcast", [P, NBLK * P], dt.float32, kind="ExternalInput")
    t_dbk = nc.dram_tensor("dinv_blk", [P, NBLK], dt.float32, kind="ExternalInput")
    t_W = [nc.dram_tensor("W1", [F, H], dt.float32, kind="ExternalInput"),
           nc.dram_tensor("W2", [H, H], dt.float32, kind="ExternalInput"),
           nc.dram_tensor("W3", [H, O], dt.float32, kind="ExternalInput")]
    t_b = [nc.dram_tensor("b1_bc", [P, H], dt.float32, kind="ExternalInput"),
           nc.dram_tensor("b2_bc", [P, H], dt.float32, kind="ExternalInput"),
           nc.dram_tensor("b3_bc", [P, O], dt.float32, kind="ExternalInput")]
    t_out = nc.dram_tensor("out", [NLOC, O], dt.float32, kind="ExternalOutput")

    with tile.TileContext(nc) as tc:
        with (
            tc.tile_pool(name="const", bufs=1) as cpool,
            tc.tile_pool(name="gth", bufs=3) as gpool,
            tc.tile_pool(name="smat", bufs=3) as spool,
            tc.tile_pool(name="work", bufs=3) as wpool,
            tc.tile_pool(name="hout", bufs=3) as hpool,
            tc.tile_pool(name="psA", bufs=2, space="PSUM") as psA,
            tc.tile_pool(name="psH", bufs=2, space="PSUM") as psH,
            tc.tile_pool(name="dram", bufs=1, space="DRAM") as dpool,
        ):
            # constants
            idx_t = cpool.tile([P, CT * 8], dt.int16, tag="idx")
            nc.sync.dma_start(idx_t[:], t_idx[:, :])
            dbc_t = cpool.tile([P, NBLK * P], dt.float32, tag="dbc")
            nc.sync.dma_start(dbc_t[:], t_dbc[:, :])
            dbk_t = cpool.tile([P, NBLK], dt.float32, tag="dbk")
            nc.sync.dma_start(dbk_t[:], t_dbk[:, :])
            W_t = []
            b_t = []
            for l in range(3):
                wt = cpool.tile([128, t_W[l].shape[1]], dt.float32, tag=f"W{l}")
                nc.sync.dma_start(wt[:], t_W[l][:, :])
                W_t.append(wt)
                bt = cpool.tile([P, t_b[l].shape[1]], dt.float32, tag=f"b{l}")
                nc.sync.dma_start(bt[:], t_b[l][:, :])
                b_t.append(bt)

            # inter-layer tables (internal DRAM)
            cc_in = [dpool.tile([NLOC, H], dt.float16, tag=f"ccin{l}", name=f"ccin{l}")
                     for l in range(2)]
            cc_out = [dpool.tile([N, H], dt.float16, tag=f"ccout{l}", name=f"ccout{l}")
                      for l in range(2)]

            for rep in range(DBG_REPEAT):
              for l in range(DBG_LAYERS):
                if l == 0:
                    tab_lo = t_xt[:, :]
                    tab_hi = t_xt[HI_BASE:N, :]
                else:
                    tab_lo = cc_out[l - 1][:, :]
                    tab_hi = cc_out[l - 1][HI_BASE:N, :]
                Hout = H if l < 2 else O

                for b in range(DBG_BLOCKS):
                    Cb, Ab = C[b], A[b]
                    cb0 = chunk_base[b]
                    g2 = gpool.tile([P, Cb * P], dt.float16, tag="g")
                    g3 = g2[:, :].rearrange("p (c d) -> p c d", d=P)
                    if DBG_LINGATHER:
                        nc.sync.dma_start(g2[:], t_S[:, cb0 * P:(cb0 + Cb) * P])
                    else:
                        nc.gpsimd.dma_gather(
                            out_ap=g3[:, 0:Ab, :],
                            in_ap=tab_lo,
                            idxs_ap=idx_t[:, cb0 * 8:(cb0 + Ab) * 8],
                            num_idxs=Ab * P,
                            num_idxs_reg=Ab * P,
                            elem_size=P,
                            single_packet=False,
                        )
                        if Cb > Ab:
                            nc.gpsimd.dma_gather(
                                out_ap=g3[:, Ab:Cb, :],
                                in_ap=tab_hi,
                                idxs_ap=idx_t[:, (cb0 + Ab) * 8:(cb0 + Cb) * 8],
                                num_idxs=(Cb - Ab) * P,
                                num_idxs_reg=(Cb - Ab) * P,
                                elem_size=P,
                                single_packet=False,
                            )
                    if DBG_GATHERONLY:
                        continue
                    S_t = spool.tile([P, Cb * P], dt.float16, tag="S")
                    nc.sync.dma_start(S_t[:], t_S[:, cb0 * P:(cb0 + Cb) * P])

                    agg_ps = psA.tile([P, P], dt.float32, tag="aggps")
                    for c in range(Cb):
                        nc.tensor.matmul(
                            out=agg_ps[:, :],
                            lhsT=g2[:, c * P:(c + 1) * P],
                            rhs=S_t[:, c * P:(c + 1) * P],
                            start=(c == 0),
                            stop=(c == Cb - 1),
                        )
                    # dst-side dinv scale at eviction (single PSUM reader)
                    agg = wpool.tile([P, P], dt.float32, tag="agg")
                    nc.vector.tensor_tensor(
                        out=agg[:], in0=agg_ps[:, :],
                        in1=dbc_t[:, b * P:(b + 1) * P], op=ALU.mult)
                    h_ps = psH.tile([P, Hout], dt.float32, tag="hps")
                    nc.tensor.matmul(out=h_ps[:, :], lhsT=agg[:], rhs=W_t[l][:, :],
                                     start=True, stop=True)
                    # epilogue
                    rows = P if b < NBLK - 1 else LAST_ROWS
                    t = wpool.tile([P, Hout], dt.float32, tag="t")
                    nc.vector.tensor_tensor(out=t[:], in0=h_ps[:, :], in1=b_t[l][:, :],
                                            op=ALU.add)
                    if l < 2:
                        m = wpool.tile([P, Hout], dt.float32, tag="m")
                        nc.vector.tensor_scalar(out=m[:], in0=t[:], scalar1=0.0,
                                                scalar2=None, op0=ALU.min)
                        e = wpool.tile([P, Hout], dt.float32, tag="e")
                        nc.scalar.activation(out=e[:], in_=m[:], func=ACTF.Exp)
                        r = wpool.tile([P, Hout], dt.float32, tag="r")
                        nc.vector.tensor_scalar(out=r[:], in0=t[:], scalar1=0.0,
                                                scalar2=-1.0, op0=ALU.max, op1=ALU.add)
                        s = wpool.tile([P, Hout], dt.float32, tag="s")
                        nc.vector.tensor_tensor(out=s[:], in0=r[:], in1=e[:], op=ALU.add)
                        ht = hpool.tile([P, Hout], dt.float16, tag="ht")
                        nc.vector.tensor_scalar(out=ht[:], in0=s[:],
                                                scalar1=dbk_t[:, b:b + 1], scalar2=None,
                                                op0=ALU.mult)
                        nc.sync.dma_start(cc_in[l][b * P:b * P + rows, :], ht[:rows, :])
                    else:
                        nc.sync.dma_start(t_out[b * P:b * P + rows, :], t[:rows, :])

                if l < 2 and l < DBG_LAYERS - 1 and DBG_AG and not DBG_GATHERONLY:
                    nc.gpsimd.collective_compute(
                        "AllGather",
                        mybir.AluOpType.bypass,
                        replica_groups=[list(range(NCORES))],
                        ins=[cc_in[l][:, :].opt()],
                        outs=[cc_out[l][:, :].opt()],
                    )
    nc.compile()
    return nc


def kernel(x, edge_index, W1, b1, W2, b2, W3, b3):
    from concourse.bass_utils import run_bass_kernel_spmd

    x = np.asarray(x)
    edge_index = np.asarray(edge_index)
    x_t, idx_arrs, S_arrs, dinv_bcast, dinv_blk, meta = _host_prep(x, edge_index)

    key = ("prog", meta["CT"], tuple(meta["C"]), tuple(meta["A"]))
    if key not in _CACHE:
        _CACHE[key] = _build_program(meta)
    nc = _CACHE[key]

    b1_bc = np.ascontiguousarray(np.broadcast_to(np.asarray(b1, np.float32)[None, :], (P, H)))
    b2_bc = np.ascontiguousarray(np.broadcast_to(np.asarray(b2, np.float32)[None, :], (P, H)))
    b3_bc = np.ascontiguousarray(np.broadcast_to(np.asarray(b3, np.float32)[None, :], (P, O)))
    W1 = np.ascontiguousarray(W1, np.float32)
    W2 = np.ascontiguousarray(W2, np.float32)
    W3 = np.ascontiguousarray(W3, np.float32)

    in_maps = []
    for c in range(NCORES):
        in_maps.append({
            "x_t": x_t,
            "idx": idx_arrs[c],
            "S": S_arrs[c],
            "dinv_bcast": dinv_bcast[c],
            "dinv_blk": dinv_blk[c],
            "W1": W1, "W2": W2, "W3": W3,
            "b1_bc": b1_bc, "b2_bc": b2_bc, "b3_bc": b3_bc,
        })
    res = run_bass_kernel_spmd(nc, in_maps, core_ids=list(range(NCORES)))
    out = np.concatenate([res.results[c]["out"] for c in range(NCORES)], axis=0)
    return out.astype(np.float32)
